# revision 1
# baseline (speedup 1.0000x reference)
"""Trainium2 Bass kernel for nn_PsiModel2d_83202106458323.

Computes, for N=4194304 particles with F in R^{N x 2 x 2}:
    C = F^T F; tr = trace(C); delta = sqrt(max(tr^2 - 4 det C, 1e-8))
    sigma = 0.5 (tr +- delta);  out = MLP_{2-16-16-16-1}(sigma1, sigma2)

Distribution: pure data parallel over 8 NeuronCores (N/8 particles each,
weights replicated). Inside each core:

  - particles stream through SBUF in spans of 128*T (T=256) particle-major
    tiles [128 partitions, 4T]
  - elementwise preamble on DVE/ACT/GPSIMD computes
      p = (a+d)^2 + (b-c)^2,  m = (a-d)^2 + (b+c)^2   (a,b,c,d = F entries)
      delta = sqrt(p*m + eps)
    using tr = (p+m)/2; the first MLP layer absorbs the 0.5 factors, so
    (p, m, delta) are the only features needed (sigmas never materialized)
  - a DVE 32x32 block transpose moves (p, m, delta, pad) onto partitions:
      R[32i + 4t_sub + f, 32b + j] = feature f of particle (q=32i+j,
      t=8b+t_sub); a matmul column then carries 8 particles
  - the 4 MLP layers run as full-height K=128 float32r matmuls
    (1 column/cycle; plain fp32 is 4x slower; quadrant tile_position
    concurrency hard-faults this stack): L1 uses one zero-padded stationary
    per origin strip, L2/L3 use blockdiag(8 x W), L4 accumulates 4 sparse
    stationaries into one PSUM tile
  - relu+bias is fused into the PSUM->SBUF evacuation, alternating between
    DVE tensor_scalar and ACT activation to use both engines
  - an inverse 32x32 block transpose restores particle-major layout for a
    clean contiguous output DMA

All weight/bias stationaries are laid out host-side in pack_weights and
shipped as one [128, 1288] fp32 input.
"""
import sys

sys.path.insert(0, "/opt/trn_rl_repo")
import numpy as np
import concourse.bass as bass
import concourse.tile as tile
from concourse import mybir
from concourse.vector_clock import ScopedClock

FP = mybir.dt.float32
FPR = mybir.dt.float32r
NCORES = 8
NW = 1288          # wpack columns
T_DEF = 256        # particles per partition per span
NSPANS_DEF = 16    # spans per core; per-core N = 128 * T * nspans


class TC(tile.TileContext):
    """TileContext whose final drain splits sem waits across NOPs (the nix
    walrus rejects instructions carrying more than one sync wait)."""

    def _drain_and_barrier(self, tick_clock, wait_clock):
        nc = self.nc
        collector = nc.sync.nop(nofuse=True)
        wait_clock.add_sem_waits(
            collector.ins, ScopedClock({None: tick_clock.global_clock})
        )
        si = collector.ins.sync_info
        waits = list(si.on_wait) if si is not None else []
        if si is not None and len(waits) > 1:
            si.on_wait = waits[:1]
            for w in waits[1:]:
                extra = nc.sync.nop(nofuse=True)
                extra.ins.sync_info = mybir.SyncInfo(on_wait=[w], on_update=[])
        nc.sync.drain()
        nc.all_engine_barrier()
        popped = nc._tile_sem_poison_stack.pop()
        assert popped is self._sem_poison
        nc.clear_and_free_semaphores(list(self.sems.allocated().values()))
        nc.all_engine_barrier()


def split_sync_waits(nc, max_waits=1):
    """Move excess per-instruction sync waits onto NOPs inserted just before
    the offending instruction on the same engine (same-engine program order
    preserves semantics)."""
    for fn in nc.m.functions:
        for blk in fn.blocks:
            i = 0
            while i < len(blk.instructions):
                inst = blk.instructions[i]
                si = getattr(inst, "sync_info", None)
                if si is not None and len(si.on_wait) > max_waits:
                    waits = list(si.on_wait)
                    si.on_wait = waits[:max_waits]
                    extra = waits[max_waits:]
                    ninserted = 0
                    while extra:
                        chunk, extra = extra[:max_waits], extra[max_waits:]
                        nop = mybir.InstNoOp(
                            name=nc.get_next_instruction_name(), ins=[], outs=[]
                        )
                        nop.engine = inst.engine
                        nop.sync_info = mybir.SyncInfo(on_wait=chunk, on_update=[])
                        nc.register_instruction(nop)
                        blk.instructions.insert(i, nop)
                        ninserted += 1
                    i += ninserted
                i += 1


def pack_weights(W1, b1, W2, b2, W3, b3, W4, b4):
    """Host-side stationary/bias layouts -> one [128, NW] fp32 array.

    cols    0:512  L1stat[i] (i=0..3): [32i + 4t_sub + f, 16t_sub + u];
                   f=0,1 -> (W1[0]+W1[1])[u]/4, f=2 -> (W1[0]-W1[1])[u]/2
    cols  512:640  W2stat: blockdiag 8x W2 at [16s+u, 16s+v]
    cols  640:768  W3stat: blockdiag 8x W3
    cols 768:1280  L4stat[i]: [16t_sub + u, 32i + t_sub] = W4[u]
    cols 1280:1285 b1, b2, b3, b4 (16-periodic / replicated), eps
    """
    wp = ((W1[0] + W1[1]) / 4.0).astype(np.float32)
    wd = ((W1[0] - W1[1]) / 2.0).astype(np.float32)
    wpack = np.zeros((128, NW), np.float32)
    for i in range(4):
        blk = wpack[:, 128 * i:128 * i + 128]
        for t_sub in range(8):
            r = 32 * i + 4 * t_sub
            blk[r + 0, 16 * t_sub:16 * t_sub + 16] = wp
            blk[r + 1, 16 * t_sub:16 * t_sub + 16] = wp
            blk[r + 2, 16 * t_sub:16 * t_sub + 16] = wd
    for s in range(8):
        wpack[16 * s:16 * s + 16, 512 + 16 * s:512 + 16 * s + 16] = W2
        wpack[16 * s:16 * s + 16, 640 + 16 * s:640 + 16 * s + 16] = W3
    for i in range(4):
        blk = wpack[:, 768 + 128 * i:768 + 128 * i + 128]
        for t_sub in range(8):
            blk[16 * t_sub:16 * t_sub + 16, 32 * i + t_sub] = W4[:, 0]
    wpack[:, 1280] = np.tile(b1, 8)
    wpack[:, 1281] = np.tile(b2, 8)
    wpack[:, 1282] = np.tile(b3, 8)
    wpack[:, 1283] = b4[0]
    wpack[:, 1284] = 1e-8  # EPS bias for the Sqrt activation
    return wpack


def build_program(T=T_DEF, nspans=NSPANS_DEF, mm_dtype=FPR, num_devices=NCORES):
    """Build the per-core Bass program. Per-core N = 128*T*nspans."""
    W = 4 * T          # SBUF free width of particle-major tiles
    CW = min(512, W)   # matmul moving-operand chunk width
    ncc = W // CW
    assert W % CW == 0 and T % 8 == 0

    nc = bass.Bass("TRN2", target_bir_lowering=False, debug=False,
                   num_devices=num_devices)
    f_in = nc.dram_tensor("f", [nspans, 128, W], FP, kind="ExternalInput").ap()
    wp_in = nc.dram_tensor("wpack", [128, NW], FP, kind="ExternalInput").ap()
    out_d = nc.dram_tensor("out", [nspans, 128, T], FP,
                           kind="ExternalOutput").ap()

    add, mx, sub, mult = (mybir.AluOpType.add, mybir.AluOpType.max,
                          mybir.AluOpType.subtract, mybir.AluOpType.mult)
    Relu = mybir.ActivationFunctionType.Relu
    Sqrt = mybir.ActivationFunctionType.Sqrt
    Square = mybir.ActivationFunctionType.Square

    with TC(nc) as tc:
        with (
            tc.tile_pool(name="const", bufs=1) as constp,
            tc.tile_pool(name="io", bufs=3) as iop,
            tc.tile_pool(name="mid", bufs=2) as midp,
            tc.tile_pool(name="acts", bufs=2) as actp,
            tc.tile_pool(name="ps", bufs=3, space="PSUM") as psp,
            tc.tile_pool(name="ps4", bufs=1, space="PSUM") as ps4p,
        ):
            wsb = constp.tile([128, NW], FP)
            nc.sync.dma_start(wsb[:, :], wp_in)
            wsr = constp.tile([128, 1280], mm_dtype)
            nc.vector.tensor_copy(wsr[:, :], wsb[:, 0:1280])
            b1v = wsb[:, 1280:1281]
            b2v = wsb[:, 1281:1282]
            b3v = wsb[:, 1282:1283]
            b4v = wsb[:, 1283:1284]
            epsv = wsb[:, 1284:1285]

            for sp in range(nspans):
                X = iop.tile([128, W], FP, tag="X")
                nc.sync.dma_start(X[:, :], f_in[sp])
                X4 = X.rearrange("p (t k) -> p t k", k=4)

                U = midp.tile([128, W], FP, tag="U")
                U4 = U.rearrange("p (t k) -> p t k", k=4)
                nc.gpsimd.tensor_tensor(U4[:, :, 0], X4[:, :, 0], X4[:, :, 3], add)
                nc.gpsimd.tensor_tensor(U4[:, :, 1], X4[:, :, 1], X4[:, :, 2], sub)
                nc.gpsimd.tensor_tensor(U4[:, :, 2], X4[:, :, 0], X4[:, :, 3], sub)
                nc.gpsimd.tensor_tensor(U4[:, :, 3], X4[:, :, 1], X4[:, :, 2], add)

                V = midp.tile([128, W], FP, tag="V")
                nc.scalar.activation(V[:, :], U[:, :], Square)
                V4 = V.rearrange("p (t k) -> p t k", k=4)

                G = midp.tile([128, W], FP, tag="G")
                G4 = G.rearrange("p (t k) -> p t k", k=4)
                nc.vector.tensor_tensor(G4[:, :, 0], V4[:, :, 0], V4[:, :, 1], add)
                nc.vector.tensor_tensor(G4[:, :, 1], V4[:, :, 2], V4[:, :, 3], add)
                PM = midp.tile([128, T], FP, tag="PM")
                nc.vector.tensor_tensor(PM[:, :], G4[:, :, 0], G4[:, :, 1], mult)
                nc.scalar.activation(G4[:, :, 2], PM[:, :], Sqrt, bias=epsv)
                nc.gpsimd.memset(G4[:, :, 3], 0.0)

                Rf = midp.tile([128, W], FP, tag="Rf")
                nc.vector.transpose(Rf[:, :], G[:, :])
                # fp32r matmul inputs must come from an fp32r-emitting op and
                # the DVE transpose cannot emit fp32r; GPSIMD (otherwise idle)
                # does the rounding copy.
                R = midp.tile([128, W], mm_dtype, tag="R")
                nc.gpsimd.tensor_copy(R[:, :], Rf[:, :])

                H1 = actp.tile([128, 4 * W], mm_dtype, tag="H1")
                H2 = actp.tile([128, 4 * W], mm_dtype, tag="H2")
                H3 = actp.tile([128, 4 * W], mm_dtype, tag="H3")
                H1r = H1.rearrange("p (a w) -> p a w", w=W)
                H2r = H2.rearrange("p (a w) -> p a w", w=W)
                H3r = H3.rearrange("p (a w) -> p a w", w=W)

                def evac(ps_t, Hr, g, cc, bias):
                    """relu(psum + bias) -> H[:, {2g,2g+1}, CW*cc:+CW]."""
                    src = ps_t.rearrange("p (s c) -> p s c", c=CW)
                    dst = Hr[:, 2 * g:2 * g + 2, CW * cc:CW * cc + CW]
                    if (g + cc) % 2 == 0:
                        nc.vector.tensor_scalar(dst, src, bias, 0.0, add, mx)
                    else:
                        nc.scalar.activation(dst, src, Relu, bias=bias)

                def layer(lhs_col_of, rhs_of, Hr, bias, cc, lname):
                    ps = [psp.tile([128, 2 * CW], FP, tag="ps",
                                   name=f"{lname}_{sp}_{cc}_{g}")
                          for g in range(2)]
                    for i in range(4):
                        nc.tensor.matmul(
                            ps[i // 2][:, CW * (i % 2):CW * (i % 2) + CW],
                            lhs_col_of(i), rhs_of(i),
                            start=True, stop=True,
                        )
                    for g in range(2):
                        evac(ps[g], Hr, g, cc, bias)

                for cc in range(ncc):
                    layer(lambda i: wsr[:, 128 * i:128 * i + 128],
                          lambda i: R[:, CW * cc:CW * cc + CW],
                          H1r, b1v, cc, "l1")
                    layer(lambda i: wsr[:, 512:640],
                          lambda i: H1[:, W * i + CW * cc:W * i + CW * cc + CW],
                          H2r, b2v, cc, "l2")
                    layer(lambda i: wsr[:, 640:768],
                          lambda i: H2[:, W * i + CW * cc:W * i + CW * cc + CW],
                          H3r, b3v, cc, "l3")

                # ---- L4: 4 accumulating full-height matmuls per chunk ----
                O1 = iop.tile([128, W], FP, tag="O1")
                ps4 = ps4p.tile([128, CW * ncc], FP, tag="ps4")
                for cc in range(ncc):
                    for i in range(4):
                        nc.tensor.matmul(
                            ps4[:, CW * cc:CW * cc + CW],
                            wsr[:, 768 + 128 * i:768 + 128 * i + 128],
                            H3[:, W * i + CW * cc:W * i + CW * cc + CW],
                            start=(i == 0), stop=(i == 3),
                        )
                nc.vector.tensor_scalar(O1[:, :], ps4[:, :], b4v, None, add)

                O2 = iop.tile([128, W], FP, tag="O2")
                nc.vector.transpose(O2[:, :], O1[:, :])
                osrc = O2.rearrange("p (b g) -> p b g", g=32)[:, :, 0:8]
                odst = out_d[sp].rearrange("p (b g) -> p b g", g=8)
                nc.sync.dma_start(odst, osrc)

    split_sync_waits(nc)
    return nc


_CACHE = {}


def _get_program(T, nspans):
    key = (T, nspans)
    if key not in _CACHE:
        _CACHE[key] = build_program(T, nspans)
    return _CACHE[key]


def make_in_maps(F, W1, b1, W2, b2, W3, b3, W4, b4, T=T_DEF, nspans=NSPANS_DEF):
    Fr = np.ascontiguousarray(F, dtype=np.float32).reshape(-1, 4)
    ncore = 128 * T * nspans
    assert Fr.shape[0] == ncore * NCORES
    wpack = pack_weights(
        np.asarray(W1, np.float32), np.asarray(b1, np.float32),
        np.asarray(W2, np.float32), np.asarray(b2, np.float32),
        np.asarray(W3, np.float32), np.asarray(b3, np.float32),
        np.asarray(W4, np.float32), np.asarray(b4, np.float32))
    return [
        {"f": Fr[c * ncore:(c + 1) * ncore].reshape(nspans, 128, 4 * T),
         "wpack": wpack}
        for c in range(NCORES)
    ]


def kernel(F, W1, b1, W2, b2, W3, b3, W4, b4):
    """Full-input entry point: shard across 8 NeuronCores, run, gather."""
    from concourse.bass_utils import run_bass_kernel_spmd

    T, nspans = T_DEF, NSPANS_DEF
    nc = _get_program(T, nspans)
    in_maps = make_in_maps(F, W1, b1, W2, b2, W3, b3, W4, b4, T, nspans)
    res = run_bass_kernel_spmd(nc, in_maps, core_ids=list(range(NCORES)),
                               trace=False)
    out = np.concatenate(
        [res.results[c]["out"].reshape(-1) for c in range(NCORES)])
    return out.reshape(-1, 1).astype(np.float32)



# revision 3
# speedup vs baseline: 6150.7672x; 6150.7672x over previous
"""Trainium2 Bass kernel for nn_PsiModel2d_83202106458323 — v3.

Computes, for N=4194304 particles with F in R^{N x 2 x 2}:
    C = F^T F; tr = trace(C); delta = sqrt(max(tr^2 - 4 det C, 1e-8))
    sigma = 0.5 (tr +- delta);  out = MLP_{2-16-16-16-1}(sigma1, sigma2)

Key reformulation vs the earlier kernel: the first layer only needs TWO
features per particle,
    feat0 = 2*tr = p + m,  feat1 = delta = sqrt(p*m + eps)
      (p = (a+d)^2 + (b-c)^2, m = (a-d)^2 + (b+c)^2)
since sigma1*W1[0] + sigma2*W1[1] = tr*(W1[0]+W1[1])/2 + delta*(W1[0]-W1[1])/2.
That halves the feature-transpose volume and removes the pad/memset.

Per core (data parallel over 8 cores), per span of 128*T particles:
  - planar elementwise preamble on DVE/ACT/GPSIMD (u-planes, squares on ACT,
    p, m, pm, 2tr, sqrt) producing an interleaved (2tr, delta) tile TD
  - one DVE 32x32 block transpose TD -> R [128, 2T] fp32r
    (column 32b+j holds 64 particles: 4 strips i x 16 t_sub, 2 feats each)
  - L1: 8 stationaries (strip i = g//2, t_sub half g%2) x full-width matmuls
    -> H1 g-major [128, 8*2T]; L2/L3: blockdiag(8xW) matmuls per g-block
  - PSUM->SBUF evacuation fused with bias+relu, round-robined across
    DVE tensor_scalar / ACT activation / GPSIMD tensor_scalar
  - L4: 16 sparse stationaries (g x b-parity) accumulate into a DENSE
    [128, T] psum laid out so psum4[32i + r', 32c + j] = out(32i+j, 32c+r');
    b4 is added by one extra ones-row matmul; a single [128, T] DVE block
    transpose evacuates psum4 straight into the contiguous output tile
  - contiguous output DMA (1KB descriptors)

All stationaries/biases are packed host-side into one [128, NW] fp32 input.
"""
import sys

sys.path.insert(0, "/opt/trn_rl_repo")
import numpy as np
import concourse.bass as bass
import concourse.tile as tile
from concourse import mybir
from concourse.vector_clock import ScopedClock

FP = mybir.dt.float32
FPR = mybir.dt.float32r
NCORES = 8
T_DEF = 256        # particles per partition per span
NSPANS_DEF = 16    # spans per core; per-core N = 128 * T * nspans

# wpack column map (see pack_weights); wpack is fp32r end-to-end (same bits
# as fp32 — declared fp32r in DRAM so no on-chip conversion copy is needed)
C_L1 = 0           # 8 x 128
C_B4ROW = 1024     # [1, 128] b4 row (partition 0)
C_ONES = 1152      # [1, T] ones row (partition 0) — first DMA ends at C_W2
C_W2 = lambda T: 1152 + T   # 128
C_W3 = lambda T: 1280 + T   # 128
C_L4 = lambda T: 1408 + T   # 16 x 128
NW_OF = lambda T: 3456 + T

# evacuation engine schedule: 12 evacs per span (4 per hidden layer),
# 'A' = ACT activation, 'D' = DVE tensor_scalar, 'P' = GPSIMD tensor_scalar
EVAC_SCHED = "ADAD ADAD ADAD".replace(" ", "")


class TC(tile.TileContext):
    """TileContext whose final drain splits sem waits across NOPs (the nix
    walrus rejects instructions carrying more than one sync wait)."""

    def _drain_and_barrier(self, tick_clock, wait_clock):
        nc = self.nc
        collector = nc.sync.nop(nofuse=True)
        wait_clock.add_sem_waits(
            collector.ins, ScopedClock({None: tick_clock.global_clock})
        )
        si = collector.ins.sync_info
        waits = list(si.on_wait) if si is not None else []
        if si is not None and len(waits) > 1:
            si.on_wait = waits[:1]
            for w in waits[1:]:
                extra = nc.sync.nop(nofuse=True)
                extra.ins.sync_info = mybir.SyncInfo(on_wait=[w], on_update=[])
        nc.sync.drain()
        nc.all_engine_barrier()
        popped = nc._tile_sem_poison_stack.pop()
        assert popped is self._sem_poison
        nc.clear_and_free_semaphores(list(self.sems.allocated().values()))
        nc.all_engine_barrier()


def split_sync_waits(nc, max_waits=1):
    """Move excess per-instruction sync waits onto NOPs inserted just before
    the offending instruction on the same engine (same-engine program order
    preserves semantics)."""
    for fn in nc.m.functions:
        for blk in fn.blocks:
            i = 0
            while i < len(blk.instructions):
                inst = blk.instructions[i]
                si = getattr(inst, "sync_info", None)
                if si is not None and len(si.on_wait) > max_waits:
                    waits = list(si.on_wait)
                    si.on_wait = waits[:max_waits]
                    extra = waits[max_waits:]
                    ninserted = 0
                    while extra:
                        chunk, extra = extra[:max_waits], extra[max_waits:]
                        nop = mybir.InstNoOp(
                            name=nc.get_next_instruction_name(), ins=[], outs=[]
                        )
                        nop.engine = inst.engine
                        nop.sync_info = mybir.SyncInfo(on_wait=chunk, on_update=[])
                        nc.register_instruction(nop)
                        blk.instructions.insert(i, nop)
                        ninserted += 1
                    i += ninserted
                i += 1


def pack_weights(W1, b1, W2, b2, W3, b3, W4, b4, T=T_DEF):
    """Host-side stationary layouts -> ([128, NW] fp32 wpack, [128, 4] bvec)."""
    NW = NW_OF(T)
    wt = ((W1[0] + W1[1]) / 4.0).astype(np.float32)   # applied to 2*tr
    wd = ((W1[0] - W1[1]) / 2.0).astype(np.float32)   # applied to delta
    wp = np.zeros((128, NW), np.float32)
    # L1: S1[g][32*(g//2) + 2*(8*(g%2)+s) + f, 16*s + u] = wt/wd
    for g in range(8):
        i, h = g // 2, g % 2
        blk = wp[:, C_L1 + 128 * g:C_L1 + 128 * g + 128]
        for s in range(8):
            q = 32 * i + 2 * (8 * h + s)
            blk[q + 0, 16 * s:16 * s + 16] = wt
            blk[q + 1, 16 * s:16 * s + 16] = wd
    wp[0, C_B4ROW:C_B4ROW + 128] = b4[0]
    wp[0, C_ONES:C_ONES + T] = 1.0
    # W2/W3 blockdiag
    cw2, cw3, cl4 = C_W2(T), C_W3(T), C_L4(T)
    for s in range(8):
        wp[16 * s:16 * s + 16, cw2 + 16 * s:cw2 + 16 * s + 16] = W2
        wp[16 * s:16 * s + 16, cw3 + 16 * s:cw3 + 16 * s + 16] = W3
    # L4: S4[g,P][16*s + u, 32*i + 16*P + 8*h + s] = W4[u]
    for g in range(8):
        i, h = g // 2, g % 2
        for P in range(2):
            blk = wp[:, cl4 + 128 * (2 * g + P):cl4 + 128 * (2 * g + P) + 128]
            for s in range(8):
                blk[16 * s:16 * s + 16, 32 * i + 16 * P + 8 * h + s] = W4[:, 0]
    bv = np.zeros((128, 4), np.float32)
    bv[:, 0] = np.tile(b1, 8)
    bv[:, 1] = np.tile(b2, 8)
    bv[:, 2] = np.tile(b3, 8)
    bv[:, 3] = 1e-8                # EPS bias for the Sqrt activation
    return wp, bv


def build_program(T=T_DEF, nspans=NSPANS_DEF, num_devices=NCORES,
                  evac_sched=EVAC_SCHED, psp_bufs=3, ps4_bufs=2,
                  evac_split=False, preamble_assign="PPPPPPPP"):
    """Build the per-core Bass program. Per-core N = 128*T*nspans."""
    W2T = 2 * T        # transposed tile width
    NB = W2T // 32     # 32-col blocks in R (b index range)
    assert T % 32 == 0 and NB % 2 == 0
    NW = NW_OF(T)
    CW2, CW3, CL4 = C_W2(T), C_W3(T), C_L4(T)

    nc = bass.Bass("TRN2", target_bir_lowering=False, debug=False,
                   num_devices=num_devices)
    f_in = nc.dram_tensor("f", [nspans, 128, 4 * T], FP, kind="ExternalInput").ap()
    wp_in = nc.dram_tensor("wpack", [128, NW], FPR, kind="ExternalInput").ap()
    bv_in = nc.dram_tensor("bvec", [128, 4], FP, kind="ExternalInput").ap()
    out_d = nc.dram_tensor("out", [nspans, 128, T], FP,
                           kind="ExternalOutput").ap()

    add, mx, sub, mult = (mybir.AluOpType.add, mybir.AluOpType.max,
                          mybir.AluOpType.subtract, mybir.AluOpType.mult)
    Relu = mybir.ActivationFunctionType.Relu
    Sqrt = mybir.ActivationFunctionType.Sqrt
    Square = mybir.ActivationFunctionType.Square

    with TC(nc) as tc:
        with (
            tc.tile_pool(name="const", bufs=1) as constp,
            tc.tile_pool(name="io", bufs=3) as iop,
            tc.tile_pool(name="mid", bufs=2) as midp,
            tc.tile_pool(name="r", bufs=2) as rp,
            tc.tile_pool(name="hr", bufs=3) as hrp,
            tc.tile_pool(name="acts", bufs=2) as actp,
            tc.tile_pool(name="ps", bufs=psp_bufs, space="PSUM") as psp,
            tc.tile_pool(name="ps4", bufs=ps4_bufs, space="PSUM") as ps4p,
        ):
            # fp32r end-to-end: no on-chip conversion copy. Two DMAs so the
            # L1 stationaries (+ biases) arrive quickly and the rest loads
            # behind span 0's preamble.
            wsr = constp.tile([128, NW], FPR)
            bvt = constp.tile([128, 4], FP)
            nc.sync.dma_start(wsr[:, 0:CW2], wp_in[:, 0:CW2])
            nc.sync.dma_start(bvt[:, :], bv_in)
            nc.sync.dma_start(wsr[:, CW2:NW], wp_in[:, CW2:NW])
            b1v = bvt[:, 0:1]
            b2v = bvt[:, 1:2]
            b3v = bvt[:, 2:3]
            epsv = bvt[:, 3:4]
            b4row = wsr[0:1, C_B4ROW:C_B4ROW + 128]
            ones = wsr[0:1, C_ONES:C_ONES + T]

            def evac(kind, dst, src, bias):
                if kind == "D":
                    nc.vector.tensor_scalar(dst, src, bias, 0.0, add, mx)
                elif kind == "A":
                    nc.scalar.activation(dst, src, Relu, bias=bias)
                elif kind == "M":
                    # DMA moves PSUM->SBUF raw (DMA engines are mostly idle);
                    # relu+bias then runs all-SBUF on DVE, where
                    # tensor_scalar qualifies for the 2x_2p perf mode.
                    hr = hrp.tile([128, src.shape[-1]], FP, tag="HR")
                    nc.sync.dma_start(hr[:, :], src)
                    nc.vector.tensor_scalar(dst, hr[:, :], bias, 0.0, add, mx)
                else:
                    nc.gpsimd.tensor_scalar(dst, src, bias, 0.0, add, mx)

            def stage_A(sp):
                """DMA + elementwise preamble + feature transpose -> R."""
                X = iop.tile([128, 4 * T], FP, tag="X")
                nc.sync.dma_start(X[:, :], f_in[sp])
                X4 = X.rearrange("p (t k) -> p t k", k=4)

                eng = {"D": nc.vector, "P": nc.gpsimd}
                pa = preamble_assign  # 7 chars: u0 u1 u2 u3 P M PM
                U = midp.tile([128, 4 * T], FP, tag="U")  # planar u0..u3
                eng[pa[0]].tensor_tensor(U[:, 0:T], X4[:, :, 0], X4[:, :, 3], add)
                eng[pa[1]].tensor_tensor(U[:, T:2 * T], X4[:, :, 1], X4[:, :, 2], sub)
                eng[pa[2]].tensor_tensor(U[:, 2 * T:3 * T], X4[:, :, 0], X4[:, :, 3], sub)
                eng[pa[3]].tensor_tensor(U[:, 3 * T:4 * T], X4[:, :, 1], X4[:, :, 2], add)

                V = midp.tile([128, 4 * T], FP, tag="V")
                if len(pa) > 7 and pa[7] in "DP":
                    eng[pa[7]].tensor_tensor(V[:, :], U[:, :], U[:, :], mult)
                else:
                    nc.scalar.activation(V[:, :], U[:, :], Square)

                P = midp.tile([128, T], FP, tag="P")
                M = midp.tile([128, T], FP, tag="M")
                PM = midp.tile([128, T], FP, tag="PM")
                eng[pa[4]].tensor_tensor(P[:, :], V[:, 0:T], V[:, T:2 * T], add)
                eng[pa[5]].tensor_tensor(M[:, :], V[:, 2 * T:3 * T], V[:, 3 * T:4 * T], add)
                eng[pa[6]].tensor_tensor(PM[:, :], P[:, :], M[:, :], mult)

                TD = midp.tile([128, W2T], FP, tag="TD")  # (2tr, delta) pairs
                TD2 = TD.rearrange("p (t k) -> p t k", k=2)
                nc.vector.tensor_tensor(TD2[:, :, 0], P[:, :], M[:, :], add)
                nc.scalar.activation(TD2[:, :, 1], PM[:, :], Sqrt, bias=epsv)

                # StreamTranspose cannot emit fp32r (walrus ISA check); do the
                # transpose in fp32 and convert with an all-SBUF DVE copy,
                # which qualifies for the 2x_2p perf mode.
                Rf = midp.tile([128, W2T], FP, tag="Rf")
                nc.vector.transpose(Rf[:, :], TD[:, :])
                R = rp.tile([128, W2T], FPR, tag="R")
                nc.vector.tensor_copy(R[:, :], Rf[:, :])
                return R

            def layer(sp, ev, lname, Hdst, bias, lhs_of):
                # 4 psum tiles of 2 g-blocks each
                for gg in range(4):
                    ps = psp.tile([128, 2 * W2T], FP, tag="ps",
                                  name=f"{lname}_{sp}_{gg}")
                    for g2 in range(2):
                        g = 2 * gg + g2
                        lhs, rhs = lhs_of(g)
                        nc.tensor.matmul(
                            ps[:, W2T * g2:W2T * g2 + W2T], lhs, rhs,
                            start=True, stop=True)
                    if evac_split:
                        # two half-width evacs on different engines: halves
                        # the psum-tile turnaround latency
                        for g2 in range(2):
                            evac(next(ev),
                                 Hdst[:, W2T * (2 * gg + g2):
                                      W2T * (2 * gg + g2 + 1)],
                                 ps[:, W2T * g2:W2T * g2 + W2T], bias)
                    else:
                        evac(next(ev), Hdst[:, 2 * W2T * gg:2 * W2T * (gg + 1)],
                             ps[:, :], bias)

            def stage_B1(sp, R, ev):
                """L1 matmuls + evacs -> H1."""
                H1 = actp.tile([128, 8 * W2T], FPR, tag="H1")
                layer(sp, ev, "l1", H1, b1v,
                      lambda g: (wsr[:, C_L1 + 128 * g:C_L1 + 128 * g + 128],
                                 R[:, :]))
                return H1

            def stage_B2(sp, H1, ev):
                """L2..L4 + bias + output transpose + store."""
                H2 = actp.tile([128, 8 * W2T], FPR, tag="H2")
                H3 = actp.tile([128, 8 * W2T], FPR, tag="H3")
                layer(sp, ev, "l2", H2, b2v,
                      lambda g: (wsr[:, CW2:CW2 + 128],
                                 H1[:, W2T * g:W2T * g + W2T]))
                layer(sp, ev, "l3", H3, b3v,
                      lambda g: (wsr[:, CW3:CW3 + 128],
                                 H2[:, W2T * g:W2T * g + W2T]))

                # ---- L4: dense psum accumulation + ones-row bias ----
                ps4 = ps4p.tile([128, T], FP, tag="ps4")
                H3r = H3.rearrange("p (g c P j) -> p g c P j", g=8, c=NB // 2, P=2)
                k = 0
                for g in range(8):
                    for Pb in range(2):
                        nc.tensor.matmul(
                            ps4[:, :],
                            wsr[:, CL4 + 128 * (2 * g + Pb):
                                CL4 + 128 * (2 * g + Pb) + 128],
                            H3r[:, g, :, Pb, :],
                            start=(k == 0), stop=False)
                        k += 1
                nc.tensor.matmul(ps4[:, :], b4row, ones,
                                 start=False, stop=True)

                Y = iop.tile([128, T], FP, tag="Y")
                nc.vector.transpose(Y[:, :], ps4[:, :])
                nc.sync.dma_start(out_d[sp], Y[:, :])

            # Software pipeline with a 1-span skew: the next span's preamble
            # is emitted between this span's L1 evacs and L2 matmuls, so the
            # elementwise engines interleave preamble work into the gaps the
            # PE leaves while it runs L2..L4, and R(sp+1) is ready well
            # before the PE needs it.
            R_cur = stage_A(0)
            for sp in range(nspans):
                ev = iter(evac_sched)
                H1 = stage_B1(sp, R_cur, ev)
                if sp + 1 < nspans:
                    R_cur = stage_A(sp + 1)
                stage_B2(sp, H1, ev)

    split_sync_waits(nc)
    return nc


_CACHE = {}


def _get_program(T, nspans):
    key = (T, nspans)
    if key not in _CACHE:
        _CACHE[key] = build_program(T, nspans)
    return _CACHE[key]


def make_in_maps(F, W1, b1, W2, b2, W3, b3, W4, b4, T=T_DEF, nspans=NSPANS_DEF):
    Fr = np.ascontiguousarray(F, dtype=np.float32).reshape(-1, 4)
    ncore = 128 * T * nspans
    assert Fr.shape[0] == ncore * NCORES
    wpack, bvec = pack_weights(
        np.asarray(W1, np.float32), np.asarray(b1, np.float32),
        np.asarray(W2, np.float32), np.asarray(b2, np.float32),
        np.asarray(W3, np.float32), np.asarray(b3, np.float32),
        np.asarray(W4, np.float32), np.asarray(b4, np.float32), T)
    return [
        {"f": Fr[c * ncore:(c + 1) * ncore].reshape(nspans, 128, 4 * T),
         "wpack": wpack, "bvec": bvec}
        for c in range(NCORES)
    ]


def kernel(F, W1, b1, W2, b2, W3, b3, W4, b4):
    """Full-input entry point: shard across 8 NeuronCores, run, gather."""
    from concourse.bass_utils import run_bass_kernel_spmd

    T, nspans = T_DEF, NSPANS_DEF
    nc = _get_program(T, nspans)
    in_maps = make_in_maps(F, W1, b1, W2, b2, W3, b3, W4, b4, T, nspans)
    res = run_bass_kernel_spmd(nc, in_maps, core_ids=list(range(NCORES)),
                               trace=False)
    out = np.concatenate(
        [res.results[c]["out"].reshape(-1) for c in range(NCORES)])
    return out.reshape(-1, 1).astype(np.float32)


# revision 4
# speedup vs baseline: 6242.3151x; 1.0149x over previous
"""Trainium2 Bass kernel for nn_PsiModel2d_83202106458323 — v3.

Computes, for N=4194304 particles with F in R^{N x 2 x 2}:
    C = F^T F; tr = trace(C); delta = sqrt(max(tr^2 - 4 det C, 1e-8))
    sigma = 0.5 (tr +- delta);  out = MLP_{2-16-16-16-1}(sigma1, sigma2)

Key reformulation vs the earlier kernel: the first layer only needs TWO
features per particle,
    feat0 = 2*tr = p + m,  feat1 = delta = sqrt(p*m + eps)
      (p = (a+d)^2 + (b-c)^2, m = (a-d)^2 + (b+c)^2)
since sigma1*W1[0] + sigma2*W1[1] = tr*(W1[0]+W1[1])/2 + delta*(W1[0]-W1[1])/2.
That halves the feature-transpose volume and removes the pad/memset.

Per core (data parallel over 8 cores), per span of 128*T particles:
  - planar elementwise preamble on DVE/ACT/GPSIMD (u-planes, squares on ACT,
    p, m, pm, 2tr, sqrt) producing an interleaved (2tr, delta) tile TD
  - one DVE 32x32 block transpose TD -> R [128, 2T] fp32r
    (column 32b+j holds 64 particles: 4 strips i x 16 t_sub, 2 feats each)
  - L1: 8 stationaries (strip i = g//2, t_sub half g%2) x full-width matmuls
    -> H1 g-major [128, 8*2T]; L2/L3: blockdiag(8xW) matmuls per g-block
  - PSUM->SBUF evacuation fused with bias+relu, round-robined across
    DVE tensor_scalar / ACT activation / GPSIMD tensor_scalar
  - L4: 16 sparse stationaries (g x b-parity) accumulate into a DENSE
    [128, T] psum laid out so psum4[32i + r', 32c + j] = out(32i+j, 32c+r');
    b4 is added by one extra ones-row matmul; a single [128, T] DVE block
    transpose evacuates psum4 straight into the contiguous output tile
  - contiguous output DMA (1KB descriptors)

All stationaries/biases are packed host-side into one [128, NW] fp32 input.
"""
import sys

sys.path.insert(0, "/opt/trn_rl_repo")
import numpy as np
import concourse.bass as bass
import concourse.tile as tile
from concourse import mybir
from concourse.vector_clock import ScopedClock

FP = mybir.dt.float32
FPR = mybir.dt.float32r
NCORES = 8
T_DEF = 256        # particles per partition per span
NSPANS_DEF = 16    # spans per core; per-core N = 128 * T * nspans

# wpack column map (see pack_weights); wpack is fp32r end-to-end (same bits
# as fp32 — declared fp32r in DRAM so no on-chip conversion copy is needed)
C_L1 = 0           # 8 x 128
C_B4ROW = 1024     # [1, 128] b4 row (partition 0)
C_ONES = 1152      # [1, T] ones row (partition 0) — first DMA ends at C_W2
C_W2 = lambda T: 1152 + T   # 128
C_W3 = lambda T: 1280 + T   # 128
C_L4 = lambda T: 1408 + T   # 16 x 128
NW_OF = lambda T: 3456 + T

# evacuation engine schedule: 12 evacs per span (4 per hidden layer),
# 'A' = ACT activation, 'D' = DVE tensor_scalar, 'P' = GPSIMD tensor_scalar
EVAC_SCHED = "ADAD ADAD ADAD".replace(" ", "")


class TC(tile.TileContext):
    """TileContext whose final drain splits sem waits across NOPs (the nix
    walrus rejects instructions carrying more than one sync wait)."""

    def _drain_and_barrier(self, tick_clock, wait_clock):
        nc = self.nc
        collector = nc.sync.nop(nofuse=True)
        wait_clock.add_sem_waits(
            collector.ins, ScopedClock({None: tick_clock.global_clock})
        )
        si = collector.ins.sync_info
        waits = list(si.on_wait) if si is not None else []
        if si is not None and len(waits) > 1:
            si.on_wait = waits[:1]
            for w in waits[1:]:
                extra = nc.sync.nop(nofuse=True)
                extra.ins.sync_info = mybir.SyncInfo(on_wait=[w], on_update=[])
        nc.sync.drain()
        nc.all_engine_barrier()
        popped = nc._tile_sem_poison_stack.pop()
        assert popped is self._sem_poison
        nc.clear_and_free_semaphores(list(self.sems.allocated().values()))
        nc.all_engine_barrier()


def split_sync_waits(nc, max_waits=1):
    """Move excess per-instruction sync waits onto NOPs inserted just before
    the offending instruction on the same engine (same-engine program order
    preserves semantics)."""
    for fn in nc.m.functions:
        for blk in fn.blocks:
            i = 0
            while i < len(blk.instructions):
                inst = blk.instructions[i]
                si = getattr(inst, "sync_info", None)
                if si is not None and len(si.on_wait) > max_waits:
                    waits = list(si.on_wait)
                    si.on_wait = waits[:max_waits]
                    extra = waits[max_waits:]
                    ninserted = 0
                    while extra:
                        chunk, extra = extra[:max_waits], extra[max_waits:]
                        nop = mybir.InstNoOp(
                            name=nc.get_next_instruction_name(), ins=[], outs=[]
                        )
                        nop.engine = inst.engine
                        nop.sync_info = mybir.SyncInfo(on_wait=chunk, on_update=[])
                        nc.register_instruction(nop)
                        blk.instructions.insert(i, nop)
                        ninserted += 1
                    i += ninserted
                i += 1


def pack_weights(W1, b1, W2, b2, W3, b3, W4, b4, T=T_DEF):
    """Host-side stationary layouts -> ([128, NW] fp32 wpack, [128, 4] bvec)."""
    NW = NW_OF(T)
    wt = ((W1[0] + W1[1]) / 4.0).astype(np.float32)   # applied to 2*tr
    wd = ((W1[0] - W1[1]) / 2.0).astype(np.float32)   # applied to delta
    wp = np.zeros((128, NW), np.float32)
    # L1: S1[g][32*(g//2) + 2*(8*(g%2)+s) + f, 16*s + u] = wt/wd
    for g in range(8):
        i, h = g // 2, g % 2
        blk = wp[:, C_L1 + 128 * g:C_L1 + 128 * g + 128]
        for s in range(8):
            q = 32 * i + 2 * (8 * h + s)
            blk[q + 0, 16 * s:16 * s + 16] = wt
            blk[q + 1, 16 * s:16 * s + 16] = wd
    wp[0, C_B4ROW:C_B4ROW + 128] = b4[0]
    wp[0, C_ONES:C_ONES + T] = 1.0
    # W2/W3 blockdiag
    cw2, cw3, cl4 = C_W2(T), C_W3(T), C_L4(T)
    for s in range(8):
        wp[16 * s:16 * s + 16, cw2 + 16 * s:cw2 + 16 * s + 16] = W2
        wp[16 * s:16 * s + 16, cw3 + 16 * s:cw3 + 16 * s + 16] = W3
    # L4: S4[g,P][16*s + u, 32*i + 16*P + 8*h + s] = W4[u]
    for g in range(8):
        i, h = g // 2, g % 2
        for P in range(2):
            blk = wp[:, cl4 + 128 * (2 * g + P):cl4 + 128 * (2 * g + P) + 128]
            for s in range(8):
                blk[16 * s:16 * s + 16, 32 * i + 16 * P + 8 * h + s] = W4[:, 0]
    bv = np.zeros((128, 4), np.float32)
    bv[:, 0] = np.tile(b1, 8)
    bv[:, 1] = np.tile(b2, 8)
    bv[:, 2] = np.tile(b3, 8)
    bv[:, 3] = 1e-8                # EPS bias for the Sqrt activation
    return wp, bv


def build_program(T=T_DEF, nspans=NSPANS_DEF, num_devices=NCORES,
                  evac_sched=EVAC_SCHED, psp_bufs=3, ps4_bufs=2,
                  evac_split=False, preamble_assign="PPPPPPPPPD"):
    """Build the per-core Bass program. Per-core N = 128*T*nspans."""
    W2T = 2 * T        # transposed tile width
    NB = W2T // 32     # 32-col blocks in R (b index range)
    assert T % 32 == 0 and NB % 2 == 0
    NW = NW_OF(T)
    CW2, CW3, CL4 = C_W2(T), C_W3(T), C_L4(T)

    nc = bass.Bass("TRN2", target_bir_lowering=False, debug=False,
                   num_devices=num_devices)
    f_in = nc.dram_tensor("f", [nspans, 128, 4 * T], FP, kind="ExternalInput").ap()
    wp_in = nc.dram_tensor("wpack", [128, NW], FPR, kind="ExternalInput").ap()
    bv_in = nc.dram_tensor("bvec", [128, 4], FP, kind="ExternalInput").ap()
    out_d = nc.dram_tensor("out", [nspans, 128, T], FP,
                           kind="ExternalOutput").ap()

    add, mx, sub, mult = (mybir.AluOpType.add, mybir.AluOpType.max,
                          mybir.AluOpType.subtract, mybir.AluOpType.mult)
    Relu = mybir.ActivationFunctionType.Relu
    Sqrt = mybir.ActivationFunctionType.Sqrt
    Square = mybir.ActivationFunctionType.Square

    with TC(nc) as tc:
        with (
            tc.tile_pool(name="const", bufs=1) as constp,
            tc.tile_pool(name="io", bufs=3) as iop,
            tc.tile_pool(name="mid", bufs=2) as midp,
            tc.tile_pool(name="r", bufs=2) as rp,
            tc.tile_pool(name="hr", bufs=3) as hrp,
            tc.tile_pool(name="acts", bufs=2) as actp,
            tc.tile_pool(name="ps", bufs=psp_bufs, space="PSUM") as psp,
            tc.tile_pool(name="ps4", bufs=ps4_bufs, space="PSUM") as ps4p,
        ):
            # fp32r end-to-end: no on-chip conversion copy. Two DMAs so the
            # L1 stationaries (+ biases) arrive quickly and the rest loads
            # behind span 0's preamble.
            wsr = constp.tile([128, NW], FPR)
            bvt = constp.tile([128, 4], FP)
            nc.sync.dma_start(wsr[:, 0:CW2], wp_in[:, 0:CW2])
            nc.sync.dma_start(bvt[:, :], bv_in)
            nc.sync.dma_start(wsr[:, CW2:NW], wp_in[:, CW2:NW])
            b1v = bvt[:, 0:1]
            b2v = bvt[:, 1:2]
            b3v = bvt[:, 2:3]
            epsv = bvt[:, 3:4]
            b4row = wsr[0:1, C_B4ROW:C_B4ROW + 128]
            ones = wsr[0:1, C_ONES:C_ONES + T]

            def evac(kind, dst, src, bias):
                if kind == "D":
                    nc.vector.tensor_scalar(dst, src, bias, 0.0, add, mx)
                elif kind == "A":
                    nc.scalar.activation(dst, src, Relu, bias=bias)
                elif kind == "M":
                    # DMA moves PSUM->SBUF raw (DMA engines are mostly idle);
                    # relu+bias then runs all-SBUF on DVE, where
                    # tensor_scalar qualifies for the 2x_2p perf mode.
                    hr = hrp.tile([128, src.shape[-1]], FP, tag="HR")
                    nc.sync.dma_start(hr[:, :], src)
                    nc.vector.tensor_scalar(dst, hr[:, :], bias, 0.0, add, mx)
                else:
                    nc.gpsimd.tensor_scalar(dst, src, bias, 0.0, add, mx)

            def stage_A(sp):
                """DMA + elementwise preamble + feature transpose -> R."""
                X = iop.tile([128, 4 * T], FP, tag="X")
                nc.sync.dma_start(X[:, :], f_in[sp])
                X4 = X.rearrange("p (t k) -> p t k", k=4)

                eng = {"D": nc.vector, "P": nc.gpsimd}
                pa = preamble_assign  # 7 chars: u0 u1 u2 u3 P M PM
                U = midp.tile([128, 4 * T], FP, tag="U")  # planar u0..u3
                eng[pa[0]].tensor_tensor(U[:, 0:T], X4[:, :, 0], X4[:, :, 3], add)
                eng[pa[1]].tensor_tensor(U[:, T:2 * T], X4[:, :, 1], X4[:, :, 2], sub)
                eng[pa[2]].tensor_tensor(U[:, 2 * T:3 * T], X4[:, :, 0], X4[:, :, 3], sub)
                eng[pa[3]].tensor_tensor(U[:, 3 * T:4 * T], X4[:, :, 1], X4[:, :, 2], add)

                V = midp.tile([128, 4 * T], FP, tag="V")
                if len(pa) > 7 and pa[7] in "DP":
                    eng[pa[7]].tensor_tensor(V[:, :], U[:, :], U[:, :], mult)
                else:
                    nc.scalar.activation(V[:, :], U[:, :], Square)

                # P = u0^2 + u1^2 and M = u2^2 + u3^2 fused into one 2T-wide
                # op: in0 walks (V0, V2), in1 walks (V1, V3) via a [2, 2T]
                # view of the planar V tile.
                PMp = midp.tile([128, 2 * T], FP, tag="PMp")
                PM = midp.tile([128, T], FP, tag="PM")
                Vg = V.rearrange("p (g t) -> p g t", g=2)
                eng[pa[4]].tensor_tensor(
                    PMp.rearrange("p (g t) -> p g t", g=2)[:, :, :],
                    Vg[:, :, 0:T], Vg[:, :, T:2 * T], add)
                P = PMp[:, 0:T]
                M = PMp[:, T:2 * T]
                eng[pa[6]].tensor_tensor(PM[:, :], P, M, mult)

                TD = midp.tile([128, W2T], FP, tag="TD")  # (2tr, delta) pairs
                TD2 = TD.rearrange("p (t k) -> p t k", k=2)
                td0e = eng[pa[8]] if len(pa) > 8 and pa[8] in "DP" else nc.vector
                td0e.tensor_tensor(TD2[:, :, 0], P, M, add)
                nc.scalar.activation(TD2[:, :, 1], PM[:, :], Sqrt, bias=epsv)

                # StreamTranspose cannot emit fp32r (walrus ISA check); do the
                # transpose in fp32 and convert with an all-SBUF copy (on DVE
                # this qualifies for the 2x_2p perf mode).
                Rf = midp.tile([128, W2T], FP, tag="Rf")
                nc.vector.transpose(Rf[:, :], TD[:, :])
                R = rp.tile([128, W2T], FPR, tag="R")
                rce = eng[pa[9]] if len(pa) > 9 and pa[9] in "DP" else nc.vector
                rce.tensor_copy(R[:, :], Rf[:, :])
                return R

            def layer(sp, ev, lname, Hdst, bias, lhs_of):
                # 4 psum tiles of 2 g-blocks each
                for gg in range(4):
                    ps = psp.tile([128, 2 * W2T], FP, tag="ps",
                                  name=f"{lname}_{sp}_{gg}")
                    for g2 in range(2):
                        g = 2 * gg + g2
                        lhs, rhs = lhs_of(g)
                        nc.tensor.matmul(
                            ps[:, W2T * g2:W2T * g2 + W2T], lhs, rhs,
                            start=True, stop=True)
                    if evac_split:
                        # two half-width evacs on different engines: halves
                        # the psum-tile turnaround latency
                        for g2 in range(2):
                            evac(next(ev),
                                 Hdst[:, W2T * (2 * gg + g2):
                                      W2T * (2 * gg + g2 + 1)],
                                 ps[:, W2T * g2:W2T * g2 + W2T], bias)
                    else:
                        evac(next(ev), Hdst[:, 2 * W2T * gg:2 * W2T * (gg + 1)],
                             ps[:, :], bias)

            def stage_B1(sp, R, ev):
                """L1 matmuls + evacs -> H1."""
                H1 = actp.tile([128, 8 * W2T], FPR, tag="H1")
                layer(sp, ev, "l1", H1, b1v,
                      lambda g: (wsr[:, C_L1 + 128 * g:C_L1 + 128 * g + 128],
                                 R[:, :]))
                return H1

            def stage_B2(sp, H1, ev):
                """L2..L4 + bias + output transpose + store."""
                H2 = actp.tile([128, 8 * W2T], FPR, tag="H2")
                H3 = actp.tile([128, 8 * W2T], FPR, tag="H3")
                layer(sp, ev, "l2", H2, b2v,
                      lambda g: (wsr[:, CW2:CW2 + 128],
                                 H1[:, W2T * g:W2T * g + W2T]))
                layer(sp, ev, "l3", H3, b3v,
                      lambda g: (wsr[:, CW3:CW3 + 128],
                                 H2[:, W2T * g:W2T * g + W2T]))

                # ---- L4: dense psum accumulation + ones-row bias ----
                ps4 = ps4p.tile([128, T], FP, tag="ps4")
                H3r = H3.rearrange("p (g c P j) -> p g c P j", g=8, c=NB // 2, P=2)
                k = 0
                for g in range(8):
                    for Pb in range(2):
                        nc.tensor.matmul(
                            ps4[:, :],
                            wsr[:, CL4 + 128 * (2 * g + Pb):
                                CL4 + 128 * (2 * g + Pb) + 128],
                            H3r[:, g, :, Pb, :],
                            start=(k == 0), stop=False)
                        k += 1
                nc.tensor.matmul(ps4[:, :], b4row, ones,
                                 start=False, stop=True)

                Y = iop.tile([128, T], FP, tag="Y")
                nc.vector.transpose(Y[:, :], ps4[:, :])
                nc.sync.dma_start(out_d[sp], Y[:, :])

            # Software pipeline with a 1-span skew: the next span's preamble
            # is emitted between this span's L1 evacs and L2 matmuls, so the
            # elementwise engines interleave preamble work into the gaps the
            # PE leaves while it runs L2..L4, and R(sp+1) is ready well
            # before the PE needs it.
            R_cur = stage_A(0)
            for sp in range(nspans):
                ev = iter(evac_sched)
                H1 = stage_B1(sp, R_cur, ev)
                if sp + 1 < nspans:
                    R_cur = stage_A(sp + 1)
                stage_B2(sp, H1, ev)

    split_sync_waits(nc)
    return nc


_CACHE = {}


def _get_program(T, nspans):
    key = (T, nspans)
    if key not in _CACHE:
        _CACHE[key] = build_program(T, nspans)
    return _CACHE[key]


def make_in_maps(F, W1, b1, W2, b2, W3, b3, W4, b4, T=T_DEF, nspans=NSPANS_DEF):
    Fr = np.ascontiguousarray(F, dtype=np.float32).reshape(-1, 4)
    ncore = 128 * T * nspans
    assert Fr.shape[0] == ncore * NCORES
    wpack, bvec = pack_weights(
        np.asarray(W1, np.float32), np.asarray(b1, np.float32),
        np.asarray(W2, np.float32), np.asarray(b2, np.float32),
        np.asarray(W3, np.float32), np.asarray(b3, np.float32),
        np.asarray(W4, np.float32), np.asarray(b4, np.float32), T)
    return [
        {"f": Fr[c * ncore:(c + 1) * ncore].reshape(nspans, 128, 4 * T),
         "wpack": wpack, "bvec": bvec}
        for c in range(NCORES)
    ]


def kernel(F, W1, b1, W2, b2, W3, b3, W4, b4):
    """Full-input entry point: shard across 8 NeuronCores, run, gather."""
    from concourse.bass_utils import run_bass_kernel_spmd

    T, nspans = T_DEF, NSPANS_DEF
    nc = _get_program(T, nspans)
    in_maps = make_in_maps(F, W1, b1, W2, b2, W3, b3, W4, b4, T, nspans)
    res = run_bass_kernel_spmd(nc, in_maps, core_ids=list(range(NCORES)),
                               trace=False)
    out = np.concatenate(
        [res.results[c]["out"].reshape(-1) for c in range(NCORES)])
    return out.reshape(-1, 1).astype(np.float32)
